# revision 30
# baseline (speedup 1.0000x reference)
"""Trainium2 Bass kernel: LSTM decoder benchmark (nn_DecoderRealBenchmark).

Model (per reference):
  x_t = concat(a_t, (t/100)*ones)            # (B, 128)
  gates = x_t @ W_ih.T + b_ih + h @ W_hh.T + b_hh
  i,f,g,o = split(gates); c = sig(f)*c + sig(i)*tanh(g); h = sig(o)*tanh(c)
  hs = stack of h over T
  x_hat = elu(hs @ W1.T + b1) @ W2.T + b2
Returns (x_hat, hs).

Strategy:
- Data-parallel over batch: 8 cores x 64 batch each; weights replicated.
- "Gate-chunks on partitions" layout: the 2048 gate rows live as 16
  chunks of 128 partitions, batch (64) on the free dim.  h is produced in
  exactly the layout the next step's matmul consumes (transpose-free).
- Matmuls in bf16 with fp32 PSUM accumulate; cell state fp32.
- All sigmoids are rewritten as tanh (sig(x) = (1+tanh(x/2))/2) so the
  whole kernel uses one activation-table set (exp_and_others: tanh, exp,
  relu, identity) - no per-step table reloads.  Scale factors fold into
  rescaled state Q=2c, hidden h2=2h, and host-side halved W_hh and W1:
    t_g = tanh(g);  t_x = tanh(x/2) for x in i,f,o
    u2 = (t_i + 1) * t_g          # = 2*sig(i)*tanh(g)
    w2 = (t_f + 1) * Q            # = 2*sig(f)*2c
    Q' = 0.5*w2 + u2              # = 2c'
    t_c = tanh(0.5*Q')            # = tanh(c')
    h2 = (t_o + 1) * t_c          # = 2h
- Time feature + both biases fold into a 66-row stationary x-operand:
  rows [Wa.T; b_ih+b_hh; sum_cols(W_ih[:,64:])] vs moving [a_t; 1; t/100].
- Gates permuted to [g,i,f,o]; PSUM groups are bank-granular (one
  start/stop pair per 2KB bank).
- MLP head runs inside the loop in HB-step blocks (wide matmuls, head
  weights' LDWEIGHTS amortized); b1 rides the ELU ops (ACT bias / fused
  tensor_scalar); ELU+1 = min(exp(v),1) + max(v,0) with the "-1" folded
  into b2' = b2 - W2 @ 1; the elementwise combine runs on GpSimd.
"""

import sys

import numpy as np
import ml_dtypes

sys.path.insert(0, "/opt/trn_rl_repo")

import concourse.bass as bass
import concourse.tile as tile
from concourse import bacc, mybir
from concourse.bass_utils import run_bass_kernel_spmd

FP32 = mybir.dt.float32
BF16 = mybir.dt.bfloat16
AF = mybir.ActivationFunctionType
ALU = mybir.AluOpType

T, B, L, A, O = 100, 512, 512, 64, 64
NCORES = 8
BL = B // NCORES          # 64 batch per core
NCH = (4 * L) // 128      # 16 gate chunks
LCH = L // 128            # 4 hidden chunks
KX = A + 2                # 66 rows of the x stationary operand
FPAD = 640                # head features padded 513 -> 640
FCH = FPAD // 128         # 5
HB = 4                    # head block size (timesteps)

_CACHE = {}


def bf16c(x):
    return np.ascontiguousarray(np.asarray(x, np.float32).astype(ml_dtypes.bfloat16))


def build_bass(bench_io=False):
    # bench_io=True: route the big outputs to internal DRAM (so wall-clock
    # benches measure exec, not axon output transfer); tiny probe output.
    nc = bacc.Bacc("TRN2", target_bir_lowering=False, debug=False, num_devices=NCORES)

    x_ext = nc.declare_dram_parameter("x_ext", [KX, T * BL], BF16, isOutput=False)
    wx_ext = nc.declare_dram_parameter("wx_ext", [KX, 4 * L], BF16, isOutput=False)
    wh_ext = nc.declare_dram_parameter("wh_ext", [L, 4 * L], BF16, isOutput=False)
    h0_ext = nc.declare_dram_parameter("h0_ext", [128, LCH * BL], BF16, isOutput=False)
    c0_ext = nc.declare_dram_parameter("c0_ext", [128, LCH * BL], FP32, isOutput=False)
    w1_ext = nc.declare_dram_parameter("w1_ext", [L, FPAD], BF16, isOutput=False)
    b1_ext = nc.declare_dram_parameter("b1_ext", [128, FCH], FP32, isOutput=False)
    w2_ext = nc.declare_dram_parameter("w2_ext", [FPAD, O], BF16, isOutput=False)
    b2_ext = nc.declare_dram_parameter("b2_ext", [O, 1], FP32, isOutput=False)
    if bench_io:
        hs_out = nc.dram_tensor("hs_int", [T, L, BL], FP32)
        xh_out = nc.dram_tensor("xh_int", [T, O, BL], FP32)
        probe = nc.declare_dram_parameter("probe", [128, LCH * BL], FP32, isOutput=True)
    else:
        hs_out = nc.declare_dram_parameter("hs_out", [T, L, BL], FP32, isOutput=True)
        xh_out = nc.declare_dram_parameter("xh_out", [T, O, BL], FP32, isOutput=True)
        probe = None

    with tile.TileContext(nc) as tc, bass.ExitStack() as ctx:
        const = ctx.enter_context(tc.tile_pool(name="const", bufs=1))
        work = ctx.enter_context(tc.tile_pool(name="work", bufs=2))
        hfp_pool = ctx.enter_context(tc.tile_pool(name="hfp", bufs=3))
        psum = ctx.enter_context(tc.tile_pool(name="psum", bufs=2, space="PSUM"))
        upsum = ctx.enter_context(tc.tile_pool(name="upsum", bufs=2, space="PSUM"))
        xpsum = ctx.enter_context(tc.tile_pool(name="xpsum", bufs=2, space="PSUM"))
        elu_pool = ctx.enter_context(tc.tile_pool(name="elu", bufs=2))
        xsb_pool = ctx.enter_context(tc.tile_pool(name="xsb", bufs=2))

        # ---- resident tensors ----
        wh_sb = const.tile([128, LCH * 4 * L], BF16)  # 0.5*W_hh.T, [p, l*2048+m*128+j]
        nc.sync.dma_start(
            out=wh_sb[:].rearrange("p (l m) -> p l m", l=LCH),
            in_=wh_ext[:].rearrange("(l p) m -> p l m", p=128),
        )
        wx_sb = const.tile([KX, 4 * L], BF16)
        nc.sync.dma_start(out=wx_sb[:], in_=wx_ext[:])
        x_sb = const.tile([KX, T * BL], BF16)
        nc.sync.dma_start(out=x_sb[:], in_=x_ext[:])

        hbuf = const.tile([128, (T + 1) * LCH * BL], BF16)  # h2 history
        nc.sync.dma_start(out=hbuf[:, 0 : LCH * BL], in_=h0_ext[:])
        q_sb = const.tile([128, LCH * BL], FP32)            # Q = 2c
        nc.sync.dma_start(out=q_sb[:], in_=c0_ext[:])

        w1_sb = const.tile([128, LCH * FPAD], BF16)  # 0.5*W1p.T
        nc.sync.dma_start(
            out=w1_sb[:].rearrange("p (l f) -> p l f", l=LCH),
            in_=w1_ext[:].rearrange("(l p) f -> p l f", p=128),
        )
        b1_sb = const.tile([128, FCH], FP32)   # b1_sb[:, j] = b1p[128j:128j+128]
        nc.sync.dma_start(out=b1_sb[:], in_=b1_ext[:])
        w2_sb = const.tile([128, FCH * O], BF16)
        nc.sync.dma_start(
            out=w2_sb[:].rearrange("p (l f) -> p l f", l=FCH),
            in_=w2_ext[:].rearrange("(l p) f -> p l f", p=128),
        )
        b2_sb = const.tile([O, 1], FP32)
        nc.sync.dma_start(out=b2_sb[:], in_=b2_ext[:])

        CW = LCH * BL  # 256
        hview = hbuf[:].rearrange("p (t l b) -> p t l b", t=T + 1, l=LCH)

        def x_mms(psab, t):
            # opens each bank's accumulation group (m%8==0)
            for m in range(NCH):
                ps = psab[m // 8]
                nc.tensor.matmul(
                    ps[:, (m % 8) * BL : (m % 8 + 1) * BL],
                    wx_sb[:, m * 128 : (m + 1) * 128],
                    x_sb[:, t * BL : (t + 1) * BL],
                    start=(m % 8 == 0),
                    stop=False,
                )

        def head_block(t0, nt):
            # head for steps t0..t0+nt-1 (h2 slots t0+1..t0+nt)
            n = nt * BL
            rhs = hview[:, t0 + 1 : t0 + 1 + nt, :, :]
            elus = []
            for j in range(FCH):
                up = upsum.tile([128, HB * BL], FP32, tag="up")
                for l in range(LCH):
                    nc.tensor.matmul(
                        up[:, 0:n],
                        w1_sb[:, l * FPAD + j * 128 : l * FPAD + (j + 1) * 128],
                        rhs[:, :, l, :],
                        start=(l == 0),
                        stop=(l == LCH - 1),
                    )
                # v = up + b1[j] (fused as bias); elu+1 = min(exp(v),1) + max(v,0)
                bj = b1_sb[:, j : j + 1]
                e = elu_pool.tile([128, HB * BL], BF16, tag="e")
                nc.scalar.activation(e[:, 0:n], up[:, 0:n], AF.Exp, bias=bj)
                r = elu_pool.tile([128, HB * BL], BF16, tag="r")
                nc.vector.tensor_scalar(
                    r[:, 0:n], up[:, 0:n], bj, 0.0, ALU.add, ALU.max
                )
                e2 = elu_pool.tile([128, HB * BL], BF16, tag="e2")
                nc.gpsimd.tensor_scalar_min(e2[:, 0:n], e[:, 0:n], 1.0)
                elu = elu_pool.tile([128, HB * BL], BF16, tag=f"elu{j}")
                nc.vector.tensor_tensor(elu[:, 0:n], e2[:, 0:n], r[:, 0:n], ALU.add)
                elus.append(elu)
            xp = xpsum.tile([O, HB * BL], FP32, tag="xp")
            for j in range(FCH):
                nc.tensor.matmul(
                    xp[:, 0:n], w2_sb[:, j * O : (j + 1) * O], elus[j][:, 0:n],
                    start=(j == 0), stop=(j == FCH - 1),
                )
            xs = xsb_pool.tile([O, HB * BL], FP32, tag="xs")
            nc.scalar.activation(xs[:, 0:n], xp[:, 0:n], AF.Identity, bias=b2_sb[:])
            nc.sync.dma_start(
                out=xh_out[t0 : t0 + nt].rearrange("t c b -> c t b"),
                in_=xs[:, 0:n].rearrange("c (t b) -> c t b", t=nt),
            )

        # ---------------- recurrence ----------------
        # gates PSUM split into two bank-sized tiles so the activations on
        # bank0 [g,i] can start while bank1 [f,o] matmuls still run.
        ps_a = psum.tile([128, 8 * BL], FP32, tag="psA", name="ps_a")
        ps_b = psum.tile([128, 8 * BL], FP32, tag="psB", name="ps_b")
        ps_cur = (ps_a, ps_b)
        x_mms(ps_cur, 0)

        for t in range(T):
            hslot = hbuf[:, t * CW : (t + 1) * CW]

            def h_mms(bank):
                ps = ps_cur[bank]
                for mm in range(8):
                    m = bank * 8 + mm
                    for l in range(LCH):
                        nc.tensor.matmul(
                            ps[:, mm * BL : (mm + 1) * BL],
                            wh_sb[:, l * 4 * L + m * 128 : l * 4 * L + (m + 1) * 128],
                            hslot[:, l * BL : (l + 1) * BL],
                            start=False,
                            stop=(mm == 7 and l == LCH - 1),
                        )

            h_mms(0)
            # i/f/o weight rows are pre-halved host-side, so every gate is a
            # plain tanh: one ACT op per PSUM bank.  Emitted between the two
            # banks' matmuls so it can start as soon as bank0 closes.
            tgi = work.tile([128, 2 * CW], BF16, tag="tgi")   # [tanh(g), tanh(i/2)]
            nc.scalar.activation(tgi[:], ps_cur[0][:], AF.Tanh)
            h_mms(1)
            u2 = work.tile([128, CW], BF16, tag="u2")
            nc.vector.scalar_tensor_tensor(
                u2[:], tgi[:, CW : 2 * CW], 1.0, tgi[:, 0:CW], ALU.add, ALU.mult
            )
            tf = work.tile([128, CW], BF16, tag="tf")         # tanh(f/2)
            nc.scalar.activation(tf[:], ps_cur[1][:, 0:CW], AF.Tanh)
            w2t = work.tile([128, CW], FP32, tag="w2t")
            nc.vector.scalar_tensor_tensor(
                w2t[:], tf[:], 1.0, q_sb[:], ALU.add, ALU.mult
            )
            to = work.tile([128, CW], BF16, tag="to")         # tanh(o/2)
            nc.scalar.activation(to[:], ps_cur[1][:, CW : 2 * CW], AF.Tanh)
            nc.vector.scalar_tensor_tensor(
                q_sb[:], w2t[:], 0.5, u2[:], ALU.mult, ALU.add
            )
            tch = work.tile([128, CW], BF16, tag="tc")
            nc.scalar.activation(tch[:], q_sb[:], AF.Tanh, scale=0.5)
            # h2 written in two halves so next step's l=0,1 matmuls start early
            HW2 = CW // 2
            nc.vector.scalar_tensor_tensor(
                hbuf[:, (t + 1) * CW : (t + 1) * CW + HW2], to[:, 0:HW2], 1.0,
                tch[:, 0:HW2], ALU.add, ALU.mult,
            )
            nc.vector.scalar_tensor_tensor(
                hbuf[:, (t + 1) * CW + HW2 : (t + 2) * CW], to[:, HW2:CW], 1.0,
                tch[:, HW2:CW], ALU.add, ALU.mult,
            )
            if t + 1 < T:
                ps_a = psum.tile([128, 8 * BL], FP32, tag="psA", name="ps_a")
                ps_b = psum.tile([128, 8 * BL], FP32, tag="psB", name="ps_b")
                ps_nxt = (ps_a, ps_b)
                x_mms(ps_nxt, t + 1)
            hfp = hfp_pool.tile([128, CW], FP32, tag="hfp")
            nc.gpsimd.tensor_scalar_mul(
                hfp[:], hbuf[:, (t + 1) * CW : (t + 2) * CW], 0.5
            )
            nc.sync.dma_start(
                out=hs_out[t].rearrange("(l p) b -> p l b", p=128),
                in_=hfp[:].rearrange("p (l b) -> p l b", l=LCH),
            )
            if (t + 1) % HB == 0 or t == T - 1:
                nt = ((t + 1) - 1) % HB + 1
                head_block(t + 1 - nt, nt)
            if t + 1 < T:
                ps_cur = ps_nxt

        if probe is not None:
            nc.sync.dma_start(out=probe[:], in_=q_sb[:])

    nc.compile()
    return nc


def prep_inputs(init, a, W_ih, W_hh, b_ih, b_hh, W1, b1, W2, b2):
    """Host-side weight/layout prep. Returns per-core in_maps."""
    # gate permutation [i,f,g,o] -> [g,i,f,o]
    perm = np.concatenate(
        [np.arange(2 * L, 3 * L), np.arange(0, L), np.arange(L, 2 * L),
         np.arange(3 * L, 4 * L)]
    )
    Wh = W_hh[perm]
    Wa = W_ih[perm, :A]
    w_time = W_ih[perm, A:].sum(1)
    b_tot = (b_ih + b_hh)[perm]

    # i/f/o gate rows pre-halved (exact powers of 2) so all gates are
    # tanh(x) with scale=1 on-device; g rows keep scale 1.
    rs = np.concatenate([np.ones(L, np.float32), np.full(3 * L, 0.5, np.float32)])
    wh_ext = bf16c(0.5 * Wh.T * rs[None, :])                # h2 convention
    wx_ext = bf16c(
        np.concatenate([Wa.T, b_tot[None], w_time[None]], 0) * rs[None, :]
    )

    W1p = np.zeros((FPAD, L), np.float32)
    W1p[: W1.shape[0]] = W1
    b1p = np.zeros((FPAD,), np.float32)
    b1p[: b1.shape[0]] = b1
    W2p = np.zeros((O, FPAD), np.float32)
    W2p[:, : W2.shape[1]] = W2
    b2adj = b2 - W2p.sum(1)

    w1_ext = bf16c(0.5 * W1p.T)                             # h2 convention
    b1_ext = np.ascontiguousarray(b1p.reshape(FCH, 128).T).astype(np.float32)
    w2_ext = bf16c(W2p.T)
    b2_ext = np.ascontiguousarray(b2adj.reshape(O, 1)).astype(np.float32)

    times = np.arange(T, dtype=np.float32) / np.float32(100.0)

    in_maps = []
    for k in range(NCORES):
        sl = slice(k * BL, (k + 1) * BL)
        a_loc = a[:, sl, :]
        x = np.empty((KX, T * BL), np.float32)
        x[:A] = a_loc.transpose(2, 0, 1).reshape(A, T * BL)
        x[A] = 1.0
        x[A + 1] = np.repeat(times, BL)
        h0 = init[sl].T
        h0r = np.ascontiguousarray(
            h0.reshape(LCH, 128, BL).transpose(1, 0, 2).reshape(128, LCH * BL)
        )
        in_maps.append(
            {
                "x_ext": bf16c(x),
                "wx_ext": wx_ext,
                "wh_ext": wh_ext,
                "h0_ext": bf16c(2.0 * h0r),                 # h2 = 2h
                "c0_ext": (2.0 * h0r).astype(np.float32),   # Q = 2c
                "w1_ext": w1_ext,
                "b1_ext": b1_ext,
                "w2_ext": w2_ext,
                "b2_ext": b2_ext,
            }
        )
    return in_maps


def kernel(init, a, s, W_ih, W_hh, b_ih, b_hh, W1, b1, W2, b2, **_):
    init = np.asarray(init, np.float32)
    a = np.asarray(a, np.float32)
    args = [np.asarray(x, np.float32) for x in (W_ih, W_hh, b_ih, b_hh, W1, b1, W2, b2)]
    in_maps = prep_inputs(init, a, *args)

    if "nc" not in _CACHE:
        _CACHE["nc"] = build_bass()
    nc = _CACHE["nc"]

    out = run_bass_kernel_spmd(nc, in_maps, list(range(NCORES)))
    _CACHE["last_result"] = out
    res = out.results

    x_hat = np.empty((T, B, O), np.float32)
    hs = np.empty((T, B, L), np.float32)
    for k in range(NCORES):
        sl = slice(k * BL, (k + 1) * BL)
        hs[:, sl, :] = res[k]["hs_out"].transpose(0, 2, 1)
        x_hat[:, sl, :] = res[k]["xh_out"].transpose(0, 2, 1)
    return x_hat, hs


# revision 34
# speedup vs baseline: 1.1114x; 1.1114x over previous
"""Trainium2 Bass kernel: LSTM decoder benchmark (nn_DecoderRealBenchmark).

Model (per reference):
  x_t = concat(a_t, (t/100)*ones)            # (B, 128)
  gates = x_t @ W_ih.T + b_ih + h @ W_hh.T + b_hh
  i,f,g,o = split(gates); c = sig(f)*c + sig(i)*tanh(g); h = sig(o)*tanh(c)
  hs = stack of h over T
  x_hat = elu(hs @ W1.T + b1) @ W2.T + b2
Returns (x_hat, hs).

Strategy:
- Data-parallel over batch: 8 cores x 64 batch each; weights replicated.
- "Gate-chunks on partitions" layout: the 2048 gate rows live as 16
  chunks of 128 partitions, batch (64) on the free dim.  h is produced in
  exactly the layout the next step's matmul consumes (transpose-free).
- Matmuls in bf16 with fp32 PSUM accumulate; cell state fp32.
- All sigmoids are rewritten as tanh (sig(x) = (1+tanh(x/2))/2) so the
  whole kernel uses one activation-table set (exp_and_others: tanh, exp,
  relu, identity) - no per-step table reloads.  Scale factors fold into
  rescaled state Q=2c, hidden h2=2h, and host-side halved W_hh and W1:
    t_g = tanh(g);  t_x = tanh(x/2) for x in i,f,o
    u2 = (t_i + 1) * t_g          # = 2*sig(i)*tanh(g)
    w2 = (t_f + 1) * Q            # = 2*sig(f)*2c
    Q' = 0.5*w2 + u2              # = 2c'
    t_c = tanh(0.5*Q')            # = tanh(c')
    h2 = (t_o + 1) * t_c          # = 2h
- Time feature + both biases fold into a 66-row stationary x-operand:
  rows [Wa.T; b_ih+b_hh; sum_cols(W_ih[:,64:])] vs moving [a_t; 1; t/100].
- Gates permuted to [g,i,f,o]; PSUM groups are bank-granular (one
  start/stop pair per 2KB bank).
- MLP head is spread across the loop: 5-step blocks, ONE feature-chunk
  slice per step (N=320 matmuls; head weights' LDWEIGHTS amortized), so
  head work hides in the recurrence's idle engine slots instead of
  spiking every block.  b1 rides the ELU ops (ACT bias / fused
  tensor_scalar); ELU+1 = min(exp(v),1) + max(v,0) with the "-1" folded
  into b2' = b2 - W2 @ 1; part of the elementwise work runs on GpSimd.
"""

import sys

import numpy as np
import ml_dtypes

sys.path.insert(0, "/opt/trn_rl_repo")

import concourse.bass as bass
import concourse.tile as tile
from concourse import bacc, mybir
from concourse.bass_utils import run_bass_kernel_spmd

FP32 = mybir.dt.float32
BF16 = mybir.dt.bfloat16
AF = mybir.ActivationFunctionType
ALU = mybir.AluOpType

T, B, L, A, O = 100, 512, 512, 64, 64
NCORES = 8
BL = B // NCORES          # 64 batch per core
NCH = (4 * L) // 128      # 16 gate chunks
LCH = L // 128            # 4 hidden chunks
KX = A + 2                # 66 rows of the x stationary operand
FPAD = 640                # head features padded 513 -> 640
FCH = FPAD // 128         # 5
HB = 5                    # head block size (timesteps); one j-slice per step

_CACHE = {}


def bf16c(x):
    return np.ascontiguousarray(np.asarray(x, np.float32).astype(ml_dtypes.bfloat16))


def build_bass(bench_io=False):
    # bench_io=True: route the big outputs to internal DRAM (so wall-clock
    # benches measure exec, not axon output transfer); tiny probe output.
    nc = bacc.Bacc("TRN2", target_bir_lowering=False, debug=False, num_devices=NCORES)

    x_ext = nc.declare_dram_parameter("x_ext", [KX, T * BL], BF16, isOutput=False)
    wx_ext = nc.declare_dram_parameter("wx_ext", [KX, 4 * L], BF16, isOutput=False)
    wh_ext = nc.declare_dram_parameter("wh_ext", [L, 4 * L], BF16, isOutput=False)
    h0_ext = nc.declare_dram_parameter("h0_ext", [128, LCH * BL], BF16, isOutput=False)
    c0_ext = nc.declare_dram_parameter("c0_ext", [128, LCH * BL], FP32, isOutput=False)
    w1_ext = nc.declare_dram_parameter("w1_ext", [L, FPAD], BF16, isOutput=False)
    b1_ext = nc.declare_dram_parameter("b1_ext", [128, FCH], FP32, isOutput=False)
    w2_ext = nc.declare_dram_parameter("w2_ext", [FPAD, O], BF16, isOutput=False)
    b2_ext = nc.declare_dram_parameter("b2_ext", [O, 1], FP32, isOutput=False)
    if bench_io:
        hs_out = nc.dram_tensor("hs_int", [T, L, BL], FP32)
        xh_out = nc.dram_tensor("xh_int", [T, O, BL], FP32)
        probe = nc.declare_dram_parameter("probe", [128, LCH * BL], FP32, isOutput=True)
    else:
        hs_out = nc.declare_dram_parameter("hs_out", [T, L, BL], FP32, isOutput=True)
        xh_out = nc.declare_dram_parameter("xh_out", [T, O, BL], FP32, isOutput=True)
        probe = None

    with tile.TileContext(nc) as tc, bass.ExitStack() as ctx:
        const = ctx.enter_context(tc.tile_pool(name="const", bufs=1))
        work = ctx.enter_context(tc.tile_pool(name="work", bufs=2))
        hfp_pool = ctx.enter_context(tc.tile_pool(name="hfp", bufs=3))
        psum = ctx.enter_context(tc.tile_pool(name="psum", bufs=2, space="PSUM"))
        upsum = ctx.enter_context(tc.tile_pool(name="upsum", bufs=2, space="PSUM"))
        xpsum = ctx.enter_context(tc.tile_pool(name="xpsum", bufs=2, space="PSUM"))
        elu_pool = ctx.enter_context(tc.tile_pool(name="elu", bufs=2))
        xsb_pool = ctx.enter_context(tc.tile_pool(name="xsb", bufs=2))

        # ---- resident tensors ----
        wh_sb = const.tile([128, LCH * 4 * L], BF16)  # 0.5*W_hh.T, [p, l*2048+m*128+j]
        nc.sync.dma_start(
            out=wh_sb[:].rearrange("p (l m) -> p l m", l=LCH),
            in_=wh_ext[:].rearrange("(l p) m -> p l m", p=128),
        )
        wx_sb = const.tile([KX, 4 * L], BF16)
        nc.sync.dma_start(out=wx_sb[:], in_=wx_ext[:])
        x_sb = const.tile([KX, T * BL], BF16)
        nc.sync.dma_start(out=x_sb[:], in_=x_ext[:])

        hbuf = const.tile([128, (T + 1) * LCH * BL], BF16)  # h2 history
        nc.sync.dma_start(out=hbuf[:, 0 : LCH * BL], in_=h0_ext[:])
        q_sb = const.tile([128, LCH * BL], FP32)            # Q = 2c
        nc.sync.dma_start(out=q_sb[:], in_=c0_ext[:])

        w1_sb = const.tile([128, LCH * FPAD], BF16)  # 0.5*W1p.T
        nc.sync.dma_start(
            out=w1_sb[:].rearrange("p (l f) -> p l f", l=LCH),
            in_=w1_ext[:].rearrange("(l p) f -> p l f", p=128),
        )
        b1_sb = const.tile([128, FCH], FP32)   # b1_sb[:, j] = b1p[128j:128j+128]
        nc.sync.dma_start(out=b1_sb[:], in_=b1_ext[:])
        w2_sb = const.tile([128, FCH * O], BF16)
        nc.sync.dma_start(
            out=w2_sb[:].rearrange("p (l f) -> p l f", l=FCH),
            in_=w2_ext[:].rearrange("(l p) f -> p l f", p=128),
        )
        b2_sb = const.tile([O, 1], FP32)
        nc.sync.dma_start(out=b2_sb[:], in_=b2_ext[:])

        CW = LCH * BL  # 256
        hview = hbuf[:].rearrange("p (t l b) -> p t l b", t=T + 1, l=LCH)

        def x_mms(psab, t):
            # opens each bank's accumulation group (m%8==0)
            for m in range(NCH):
                ps = psab[m // 8]
                nc.tensor.matmul(
                    ps[:, (m % 8) * BL : (m % 8 + 1) * BL],
                    wx_sb[:, m * 128 : (m + 1) * 128],
                    x_sb[:, t * BL : (t + 1) * BL],
                    start=(m % 8 == 0),
                    stop=False,
                )

        N_H = HB * BL  # 320
        cur_elus = [None] * FCH

        def head_slice(b, j):
            # feature-chunk j of the head for block b (steps 5b..5b+4)
            rhs = hview[:, HB * b + 1 : HB * b + 1 + HB, :, :]
            up = upsum.tile([128, N_H], FP32, tag="up")
            for l in range(LCH):
                nc.tensor.matmul(
                    up[:],
                    w1_sb[:, l * FPAD + j * 128 : l * FPAD + (j + 1) * 128],
                    rhs[:, :, l, :],
                    start=(l == 0),
                    stop=(l == LCH - 1),
                )
            # v = up + b1[j] (fused as bias); elu+1 = min(exp(v),1) + max(v,0)
            bj = b1_sb[:, j : j + 1]
            e = elu_pool.tile([128, N_H], BF16, tag="e")
            nc.scalar.activation(e[:], up[:], AF.Exp, bias=bj)
            r = elu_pool.tile([128, N_H], BF16, tag="r")
            nc.vector.tensor_scalar(r[:], up[:], bj, 0.0, ALU.add, ALU.max)
            e2 = elu_pool.tile([128, N_H], BF16, tag="e2")
            nc.gpsimd.tensor_scalar_min(e2[:], e[:], 1.0)
            elu = elu_pool.tile([128, N_H], BF16, tag=f"elu{j}")
            nc.vector.tensor_tensor(elu[:], e2[:], r[:], ALU.add)
            cur_elus[j] = elu

        def head_finish(b):
            t0 = HB * b
            xp = xpsum.tile([O, N_H], FP32, tag="xp")
            for j in range(FCH):
                nc.tensor.matmul(
                    xp[:], w2_sb[:, j * O : (j + 1) * O], cur_elus[j][:],
                    start=(j == 0), stop=(j == FCH - 1),
                )
            xs = xsb_pool.tile([O, N_H], FP32, tag="xs")
            nc.scalar.activation(xs[:], xp[:], AF.Identity, bias=b2_sb[:])
            nc.sync.dma_start(
                out=xh_out[t0 : t0 + HB].rearrange("t c b -> c t b"),
                in_=xs[:].rearrange("c (t b) -> c t b", t=HB),
            )

        def head_work_for(t):
            # one head slice per step: slice (b, j) after step t = 5b+4+j
            if t < HB - 1 + 0:
                return
            b, j = (t - (HB - 1)) // HB, (t - (HB - 1)) % HB
            if b >= T // HB:
                return
            head_slice(b, j)
            if j == FCH - 1:
                head_finish(b)

        # ---------------- recurrence ----------------
        # gates PSUM split into two bank-sized tiles so the activations on
        # bank0 [g,i] can start while bank1 [f,o] matmuls still run.
        ps_a = psum.tile([128, 8 * BL], FP32, tag="psA", name="ps_a")
        ps_b = psum.tile([128, 8 * BL], FP32, tag="psB", name="ps_b")
        ps_cur = (ps_a, ps_b)
        x_mms(ps_cur, 0)

        for t in range(T):
            hslot = hbuf[:, t * CW : (t + 1) * CW]

            def h_mms(bank):
                ps = ps_cur[bank]
                for mm in range(8):
                    m = bank * 8 + mm
                    for l in range(LCH):
                        nc.tensor.matmul(
                            ps[:, mm * BL : (mm + 1) * BL],
                            wh_sb[:, l * 4 * L + m * 128 : l * 4 * L + (m + 1) * 128],
                            hslot[:, l * BL : (l + 1) * BL],
                            start=False,
                            stop=(mm == 7 and l == LCH - 1),
                        )

            h_mms(0)
            # i/f/o weight rows are pre-halved host-side, so every gate is a
            # plain tanh: one ACT op per PSUM bank.  Emitted between the two
            # banks' matmuls so it can start as soon as bank0 closes.
            tgi = work.tile([128, 2 * CW], BF16, tag="tgi")   # [tanh(g), tanh(i/2)]
            nc.scalar.activation(tgi[:], ps_cur[0][:], AF.Tanh)
            h_mms(1)
            u2 = work.tile([128, CW], BF16, tag="u2")
            nc.vector.scalar_tensor_tensor(
                u2[:], tgi[:, CW : 2 * CW], 1.0, tgi[:, 0:CW], ALU.add, ALU.mult
            )
            tf = work.tile([128, CW], BF16, tag="tf")         # tanh(f/2)
            nc.scalar.activation(tf[:], ps_cur[1][:, 0:CW], AF.Tanh)
            w2t = work.tile([128, CW], FP32, tag="w2t")
            nc.vector.scalar_tensor_tensor(
                w2t[:], tf[:], 1.0, q_sb[:], ALU.add, ALU.mult
            )
            to = work.tile([128, CW], BF16, tag="to")         # tanh(o/2)
            nc.scalar.activation(to[:], ps_cur[1][:, CW : 2 * CW], AF.Tanh)
            nc.vector.scalar_tensor_tensor(
                q_sb[:], w2t[:], 0.5, u2[:], ALU.mult, ALU.add
            )
            tch = work.tile([128, CW], BF16, tag="tc")
            nc.scalar.activation(tch[:], q_sb[:], AF.Tanh, scale=0.5)
            # h2 written in two halves so next step's l=0,1 matmuls start early
            HW2 = CW // 2
            nc.vector.scalar_tensor_tensor(
                hbuf[:, (t + 1) * CW : (t + 1) * CW + HW2], to[:, 0:HW2], 1.0,
                tch[:, 0:HW2], ALU.add, ALU.mult,
            )
            nc.vector.scalar_tensor_tensor(
                hbuf[:, (t + 1) * CW + HW2 : (t + 2) * CW], to[:, HW2:CW], 1.0,
                tch[:, HW2:CW], ALU.add, ALU.mult,
            )
            if t + 1 < T:
                ps_a = psum.tile([128, 8 * BL], FP32, tag="psA", name="ps_a")
                ps_b = psum.tile([128, 8 * BL], FP32, tag="psB", name="ps_b")
                ps_nxt = (ps_a, ps_b)
                x_mms(ps_nxt, t + 1)
            hfp = hfp_pool.tile([128, CW], FP32, tag="hfp")
            nc.gpsimd.tensor_scalar_mul(
                hfp[:], hbuf[:, (t + 1) * CW : (t + 2) * CW], 0.5
            )
            nc.sync.dma_start(
                out=hs_out[t].rearrange("(l p) b -> p l b", p=128),
                in_=hfp[:].rearrange("p (l b) -> p l b", l=LCH),
            )
            head_work_for(t)
            if t + 1 < T:
                ps_cur = ps_nxt

        # drain remaining head slices (blocks whose slices extend past t=T-1)
        for t in range(T, T + 2 * HB):
            head_work_for(t)

        if probe is not None:
            nc.sync.dma_start(out=probe[:], in_=q_sb[:])

    nc.compile()
    return nc


def prep_inputs(init, a, W_ih, W_hh, b_ih, b_hh, W1, b1, W2, b2):
    """Host-side weight/layout prep. Returns per-core in_maps."""
    # gate permutation [i,f,g,o] -> [g,i,f,o]
    perm = np.concatenate(
        [np.arange(2 * L, 3 * L), np.arange(0, L), np.arange(L, 2 * L),
         np.arange(3 * L, 4 * L)]
    )
    Wh = W_hh[perm]
    Wa = W_ih[perm, :A]
    w_time = W_ih[perm, A:].sum(1)
    b_tot = (b_ih + b_hh)[perm]

    # i/f/o gate rows pre-halved (exact powers of 2) so all gates are
    # tanh(x) with scale=1 on-device; g rows keep scale 1.
    rs = np.concatenate([np.ones(L, np.float32), np.full(3 * L, 0.5, np.float32)])
    wh_ext = bf16c(0.5 * Wh.T * rs[None, :])                # h2 convention
    wx_ext = bf16c(
        np.concatenate([Wa.T, b_tot[None], w_time[None]], 0) * rs[None, :]
    )

    W1p = np.zeros((FPAD, L), np.float32)
    W1p[: W1.shape[0]] = W1
    b1p = np.zeros((FPAD,), np.float32)
    b1p[: b1.shape[0]] = b1
    W2p = np.zeros((O, FPAD), np.float32)
    W2p[:, : W2.shape[1]] = W2
    b2adj = b2 - W2p.sum(1)

    w1_ext = bf16c(0.5 * W1p.T)                             # h2 convention
    b1_ext = np.ascontiguousarray(b1p.reshape(FCH, 128).T).astype(np.float32)
    w2_ext = bf16c(W2p.T)
    b2_ext = np.ascontiguousarray(b2adj.reshape(O, 1)).astype(np.float32)

    times = np.arange(T, dtype=np.float32) / np.float32(100.0)

    in_maps = []
    for k in range(NCORES):
        sl = slice(k * BL, (k + 1) * BL)
        a_loc = a[:, sl, :]
        x = np.empty((KX, T * BL), np.float32)
        x[:A] = a_loc.transpose(2, 0, 1).reshape(A, T * BL)
        x[A] = 1.0
        x[A + 1] = np.repeat(times, BL)
        h0 = init[sl].T
        h0r = np.ascontiguousarray(
            h0.reshape(LCH, 128, BL).transpose(1, 0, 2).reshape(128, LCH * BL)
        )
        in_maps.append(
            {
                "x_ext": bf16c(x),
                "wx_ext": wx_ext,
                "wh_ext": wh_ext,
                "h0_ext": bf16c(2.0 * h0r),                 # h2 = 2h
                "c0_ext": (2.0 * h0r).astype(np.float32),   # Q = 2c
                "w1_ext": w1_ext,
                "b1_ext": b1_ext,
                "w2_ext": w2_ext,
                "b2_ext": b2_ext,
            }
        )
    return in_maps


def kernel(init, a, s, W_ih, W_hh, b_ih, b_hh, W1, b1, W2, b2, **_):
    init = np.asarray(init, np.float32)
    a = np.asarray(a, np.float32)
    args = [np.asarray(x, np.float32) for x in (W_ih, W_hh, b_ih, b_hh, W1, b1, W2, b2)]
    in_maps = prep_inputs(init, a, *args)

    if "nc" not in _CACHE:
        _CACHE["nc"] = build_bass()
    nc = _CACHE["nc"]

    out = run_bass_kernel_spmd(nc, in_maps, list(range(NCORES)))
    _CACHE["last_result"] = out
    res = out.results

    x_hat = np.empty((T, B, O), np.float32)
    hs = np.empty((T, B, L), np.float32)
    for k in range(NCORES):
        sl = slice(k * BL, (k + 1) * BL)
        hs[:, sl, :] = res[k]["hs_out"].transpose(0, 2, 1)
        x_hat[:, sl, :] = res[k]["xh_out"].transpose(0, 2, 1)
    return x_hat, hs


# revision 41
# speedup vs baseline: 1.1258x; 1.0129x over previous
"""Trainium2 Bass kernel: LSTM decoder benchmark (nn_DecoderRealBenchmark).

Model (per reference):
  x_t = concat(a_t, (t/100)*ones)            # (B, 128)
  gates = x_t @ W_ih.T + b_ih + h @ W_hh.T + b_hh
  i,f,g,o = split(gates); c = sig(f)*c + sig(i)*tanh(g); h = sig(o)*tanh(c)
  hs = stack of h over T
  x_hat = elu(hs @ W1.T + b1) @ W2.T + b2
Returns (x_hat, hs).

Strategy:
- Data-parallel over batch: 8 cores x 64 batch each; weights replicated.
- "Gate-chunks on partitions" layout: the 2048 gate rows live as 16
  chunks of 128 partitions, batch (64) on the free dim.  h is produced in
  exactly the layout the next step's matmul consumes (transpose-free).
- Matmuls in bf16 with fp32 PSUM accumulate; cell state fp32.
- All sigmoids are rewritten as tanh (sig(x) = (1+tanh(x/2))/2) so the
  whole kernel uses one activation-table set (exp_and_others: tanh, exp,
  relu, identity) - no per-step table reloads.  Scale factors fold into
  rescaled state Q=2c, hidden h2=2h, and host-side halved W_hh and W1:
    t_g = tanh(g);  t_x = tanh(x/2) for x in i,f,o
    u2 = (t_i + 1) * t_g          # = 2*sig(i)*tanh(g)
    w2 = (t_f + 1) * Q            # = 2*sig(f)*2c
    Q' = 0.5*w2 + u2              # = 2c'
    t_c = tanh(0.5*Q')            # = tanh(c')
    h2 = (t_o + 1) * t_c          # = 2h
- Time feature + both biases fold into a 66-row stationary x-operand:
  rows [Wa.T; b_ih+b_hh; sum_cols(W_ih[:,64:])] vs moving [a_t; 1; t/100].
- Gates permuted to [g,i,f,o]; PSUM groups are bank-granular (one
  start/stop pair per 2KB bank).
- MLP head is spread across the loop: 5-step blocks, ONE feature-chunk
  slice per step (N=320 matmuls; head weights' LDWEIGHTS amortized), so
  head work hides in the recurrence's idle engine slots instead of
  spiking every block.  b1 rides the ELU ops (ACT bias / fused
  tensor_scalar); ELU+1 = min(exp(v),1) + max(v,0) with the "-1" folded
  into b2' = b2 - W2 @ 1; part of the elementwise work runs on GpSimd.
"""

import sys

import numpy as np
import ml_dtypes

sys.path.insert(0, "/opt/trn_rl_repo")

import concourse.bass as bass
import concourse.tile as tile
from concourse import bacc, mybir
from concourse.bass_utils import run_bass_kernel_spmd

FP32 = mybir.dt.float32
BF16 = mybir.dt.bfloat16
AF = mybir.ActivationFunctionType
ALU = mybir.AluOpType

T, B, L, A, O = 100, 512, 512, 64, 64
NCORES = 8
BL = B // NCORES          # 64 batch per core
NCH = (4 * L) // 128      # 16 gate chunks
LCH = L // 128            # 4 hidden chunks
KX = A + 2                # 66 rows of the x stationary operand
FPAD = 640                # head features padded 513 -> 640
FCH = FPAD // 128         # 5
HB = 5                    # head block size (timesteps); one j-slice per step

_CACHE = {}


def bf16c(x):
    return np.ascontiguousarray(np.asarray(x, np.float32).astype(ml_dtypes.bfloat16))


def build_bass(bench_io=False):
    # bench_io=True: route the big outputs to internal DRAM (so wall-clock
    # benches measure exec, not axon output transfer); tiny probe output.
    nc = bacc.Bacc("TRN2", target_bir_lowering=False, debug=False, num_devices=NCORES)

    x_ext = nc.declare_dram_parameter("x_ext", [KX, T * BL], BF16, isOutput=False)
    wx_ext = nc.declare_dram_parameter("wx_ext", [KX, 4 * L], BF16, isOutput=False)
    wh_ext = nc.declare_dram_parameter("wh_ext", [L, 4 * L], BF16, isOutput=False)
    h0_ext = nc.declare_dram_parameter("h0_ext", [128, LCH * BL], BF16, isOutput=False)
    c0_ext = nc.declare_dram_parameter("c0_ext", [128, LCH * BL], FP32, isOutput=False)
    w1_ext = nc.declare_dram_parameter("w1_ext", [L, FPAD], BF16, isOutput=False)
    b1_ext = nc.declare_dram_parameter("b1_ext", [128, FCH], FP32, isOutput=False)
    w2_ext = nc.declare_dram_parameter("w2_ext", [FPAD, O], BF16, isOutput=False)
    b2_ext = nc.declare_dram_parameter("b2_ext", [O, 1], FP32, isOutput=False)
    if bench_io:
        hs_out = nc.dram_tensor("hs_int", [T, L, BL], FP32)
        xh_out = nc.dram_tensor("xh_int", [T, O, BL], FP32)
        probe = nc.declare_dram_parameter("probe", [128, LCH * BL], FP32, isOutput=True)
    else:
        hs_out = nc.declare_dram_parameter("hs_out", [T, L, BL], FP32, isOutput=True)
        xh_out = nc.declare_dram_parameter("xh_out", [T, O, BL], FP32, isOutput=True)
        probe = None

    with tile.TileContext(nc) as tc, bass.ExitStack() as ctx:
        const = ctx.enter_context(tc.tile_pool(name="const", bufs=1))
        work = ctx.enter_context(tc.tile_pool(name="work", bufs=2))
        hfp_pool = ctx.enter_context(tc.tile_pool(name="hfp", bufs=3))
        psum = ctx.enter_context(tc.tile_pool(name="psum", bufs=2, space="PSUM"))
        upsum = ctx.enter_context(tc.tile_pool(name="upsum", bufs=1, space="PSUM"))
        xpsum = ctx.enter_context(tc.tile_pool(name="xpsum", bufs=1, space="PSUM"))
        elu_pool = ctx.enter_context(tc.tile_pool(name="elu", bufs=2))
        xsb_pool = ctx.enter_context(tc.tile_pool(name="xsb", bufs=2))

        # ---- resident tensors ----
        wh_sb = const.tile([128, LCH * 4 * L], BF16)  # 0.5*W_hh.T, [p, l*2048+m*128+j]
        nc.sync.dma_start(
            out=wh_sb[:].rearrange("p (l m) -> p l m", l=LCH),
            in_=wh_ext[:].rearrange("(l p) m -> p l m", p=128),
        )
        wx_sb = const.tile([KX, 4 * L], BF16)
        nc.sync.dma_start(out=wx_sb[:], in_=wx_ext[:])
        x_sb = const.tile([KX, T * BL], BF16)
        nc.sync.dma_start(out=x_sb[:], in_=x_ext[:])

        hbuf = const.tile([128, (T + 1) * LCH * BL], BF16)  # h2 history
        nc.sync.dma_start(out=hbuf[:, 0 : LCH * BL], in_=h0_ext[:])
        q_sb = const.tile([128, LCH * BL], FP32)            # Q = 2c
        nc.sync.dma_start(out=q_sb[:], in_=c0_ext[:])

        w1_sb = const.tile([128, LCH * FPAD], BF16)  # 0.5*W1p.T
        nc.sync.dma_start(
            out=w1_sb[:].rearrange("p (l f) -> p l f", l=LCH),
            in_=w1_ext[:].rearrange("(l p) f -> p l f", p=128),
        )
        b1_sb = const.tile([128, FCH], FP32)   # b1_sb[:, j] = b1p[128j:128j+128]
        nc.sync.dma_start(out=b1_sb[:], in_=b1_ext[:])
        w2_sb = const.tile([128, FCH * O], BF16)
        nc.sync.dma_start(
            out=w2_sb[:].rearrange("p (l f) -> p l f", l=FCH),
            in_=w2_ext[:].rearrange("(l p) f -> p l f", p=128),
        )
        b2_sb = const.tile([O, 1], FP32)
        nc.sync.dma_start(out=b2_sb[:], in_=b2_ext[:])

        CW = LCH * BL  # 256
        hview = hbuf[:].rearrange("p (t l b) -> p t l b", t=T + 1, l=LCH)

        def bank_of(m):
            # gate order [f(0-3) | g,i(4-11) | o(12-15)]
            if m < 4:
                return 0, m
            if m < 12:
                return 1, m - 4
            return 2, m - 12

        def x_mms(psabc, t):
            # opens each region's accumulation group at its first chunk
            for m in range(NCH):
                bank, mm = bank_of(m)
                ps = psabc[bank]
                nc.tensor.matmul(
                    ps[:, mm * BL : (mm + 1) * BL],
                    wx_sb[:, m * 128 : (m + 1) * 128],
                    x_sb[:, t * BL : (t + 1) * BL],
                    start=(mm == 0),
                    stop=False,
                )

        N_H = HB * BL  # 320
        cur_elus = [None] * FCH

        def head_slice(b, j):
            # feature-chunk j of the head for block b (steps 5b..5b+4)
            rhs = hview[:, HB * b + 1 : HB * b + 1 + HB, :, :]
            up = upsum.tile([128, N_H], FP32, tag="up")
            for l in range(LCH):
                nc.tensor.matmul(
                    up[:],
                    w1_sb[:, l * FPAD + j * 128 : l * FPAD + (j + 1) * 128],
                    rhs[:, :, l, :],
                    start=(l == 0),
                    stop=(l == LCH - 1),
                )
            # v = up + b1[j] (fused as bias); elu+1 = min(exp(v),1) + max(v,0)
            bj = b1_sb[:, j : j + 1]
            e = elu_pool.tile([128, N_H], BF16, tag="e")
            nc.scalar.activation(e[:], up[:], AF.Exp, bias=bj)
            r = elu_pool.tile([128, N_H], BF16, tag="r")
            nc.vector.tensor_scalar(r[:], up[:], bj, 0.0, ALU.add, ALU.max)
            e2 = elu_pool.tile([128, N_H], BF16, tag="e2")
            nc.gpsimd.tensor_scalar_min(e2[:], e[:], 1.0)
            elu = elu_pool.tile([128, N_H], BF16, tag=f"elu{j}")
            nc.vector.tensor_tensor(elu[:], e2[:], r[:], ALU.add)
            cur_elus[j] = elu

        def head_finish(b):
            t0 = HB * b
            xp = xpsum.tile([O, N_H], FP32, tag="xp")
            for j in range(FCH):
                nc.tensor.matmul(
                    xp[:], w2_sb[:, j * O : (j + 1) * O], cur_elus[j][:],
                    start=(j == 0), stop=(j == FCH - 1),
                )
            xs = xsb_pool.tile([O, N_H], FP32, tag="xs")
            nc.scalar.activation(xs[:], xp[:], AF.Identity, bias=b2_sb[:])
            nc.sync.dma_start(
                out=xh_out[t0 : t0 + HB].rearrange("t c b -> c t b"),
                in_=xs[:].rearrange("c (t b) -> c t b", t=HB),
            )

        def head_work_for(t):
            # one head slice per step: slice (b, j) after step t = 5b+4+j
            if t < HB - 1 + 0:
                return
            b, j = (t - (HB - 1)) // HB, (t - (HB - 1)) % HB
            if b >= T // HB:
                return
            head_slice(b, j)
            if j == FCH - 1:
                head_finish(b)

        # ---------------- recurrence ----------------
        # gates PSUM in three regions [f | g,i | o], all double-buffered via
        # tagged pool tiles (fresh tile per step - the pattern verified on
        # HW); f-chunk matmuls are issued first so tanh(f) (the binding
        # w2->Q path) starts after 1/4 of the burst.
        def alloc_ps():
            ps_f = psum.tile([128, 4 * BL], FP32, tag="psF", name="ps_f")
            ps_gi = psum.tile([128, 8 * BL], FP32, tag="psGI", name="ps_gi")
            ps_o = psum.tile([128, 4 * BL], FP32, tag="psO", name="ps_o")
            return (ps_f, ps_gi, ps_o)

        ps_cur = alloc_ps()
        x_mms(ps_cur, 0)

        for t in range(T):
            hslot = hbuf[:, t * CW : (t + 1) * CW]

            def h_mms(mlist):
                for m in mlist:
                    bank, mm = bank_of(m)
                    ps = ps_cur[bank]
                    for l in range(LCH):
                        nc.tensor.matmul(
                            ps[:, mm * BL : (mm + 1) * BL],
                            wh_sb[:, l * 4 * L + m * 128 : l * 4 * L + (m + 1) * 128],
                            hslot[:, l * BL : (l + 1) * BL],
                            start=False,
                            stop=(l == LCH - 1
                                  and mm == (7 if bank == 1 else 3)),
                        )

            # f-chunks first: tanh(f) -> w2 path starts after 1/4 of the burst
            h_mms(range(0, 4))
            tf = work.tile([128, CW], BF16, tag="tf")         # tanh(f/2)
            nc.scalar.activation(tf[:], ps_cur[0][:], AF.Tanh)
            h_mms(range(4, 12))
            tgi = work.tile([128, 2 * CW], BF16, tag="tgi")   # [tanh(g), tanh(i/2)]
            nc.scalar.activation(tgi[:], ps_cur[1][:], AF.Tanh)
            h_mms(range(12, 16))
            w2t = work.tile([128, CW], FP32, tag="w2t")
            nc.vector.scalar_tensor_tensor(
                w2t[:], tf[:], 1.0, q_sb[:], ALU.add, ALU.mult
            )
            u2 = work.tile([128, CW], BF16, tag="u2")
            nc.vector.scalar_tensor_tensor(
                u2[:], tgi[:, CW : 2 * CW], 1.0, tgi[:, 0:CW], ALU.add, ALU.mult
            )
            to = work.tile([128, CW], BF16, tag="to")         # tanh(o/2)
            nc.scalar.activation(to[:], ps_cur[2][:], AF.Tanh)
            nc.vector.scalar_tensor_tensor(
                q_sb[:], w2t[:], 0.5, u2[:], ALU.mult, ALU.add
            )
            tch = work.tile([128, CW], BF16, tag="tc")
            nc.scalar.activation(tch[:], q_sb[:], AF.Tanh, scale=0.5)
            # h2 written in two halves so next step's l=0,1 matmuls start early
            HW2 = CW // 2
            nc.vector.scalar_tensor_tensor(
                hbuf[:, (t + 1) * CW : (t + 1) * CW + HW2], to[:, 0:HW2], 1.0,
                tch[:, 0:HW2], ALU.add, ALU.mult,
            )
            nc.vector.scalar_tensor_tensor(
                hbuf[:, (t + 1) * CW + HW2 : (t + 2) * CW], to[:, HW2:CW], 1.0,
                tch[:, HW2:CW], ALU.add, ALU.mult,
            )
            if t + 1 < T:
                ps_nxt = alloc_ps()
                x_mms(ps_nxt, t + 1)
            hfp = hfp_pool.tile([128, CW], FP32, tag="hfp")
            nc.gpsimd.tensor_scalar_mul(
                hfp[:], hbuf[:, (t + 1) * CW : (t + 2) * CW], 0.5
            )
            nc.sync.dma_start(
                out=hs_out[t].rearrange("(l p) b -> p l b", p=128),
                in_=hfp[:].rearrange("p (l b) -> p l b", l=LCH),
            )
            head_work_for(t)
            if t + 1 < T:
                ps_cur = ps_nxt

        # drain remaining head slices (blocks whose slices extend past t=T-1)
        for t in range(T, T + 2 * HB):
            head_work_for(t)

        if probe is not None:
            nc.sync.dma_start(out=probe[:], in_=q_sb[:])

    nc.compile()
    return nc


def prep_inputs(init, a, W_ih, W_hh, b_ih, b_hh, W1, b1, W2, b2):
    """Host-side weight/layout prep. Returns per-core in_maps."""
    # gate permutation [i,f,g,o] -> [f,g,i,o]
    perm = np.concatenate(
        [np.arange(L, 2 * L), np.arange(2 * L, 3 * L), np.arange(0, L),
         np.arange(3 * L, 4 * L)]
    )
    Wh = W_hh[perm]
    Wa = W_ih[perm, :A]
    w_time = W_ih[perm, A:].sum(1)
    b_tot = (b_ih + b_hh)[perm]

    # i/f/o gate rows pre-halved (exact powers of 2) so all gates are
    # tanh(x) with scale=1 on-device; g rows keep scale 1.
    rs = np.concatenate(
        [np.full(L, 0.5, np.float32), np.ones(L, np.float32),
         np.full(2 * L, 0.5, np.float32)]
    )
    wh_ext = bf16c(0.5 * Wh.T * rs[None, :])                # h2 convention
    wx_ext = bf16c(
        np.concatenate([Wa.T, b_tot[None], w_time[None]], 0) * rs[None, :]
    )

    W1p = np.zeros((FPAD, L), np.float32)
    W1p[: W1.shape[0]] = W1
    b1p = np.zeros((FPAD,), np.float32)
    b1p[: b1.shape[0]] = b1
    W2p = np.zeros((O, FPAD), np.float32)
    W2p[:, : W2.shape[1]] = W2
    b2adj = b2 - W2p.sum(1)

    w1_ext = bf16c(0.5 * W1p.T)                             # h2 convention
    b1_ext = np.ascontiguousarray(b1p.reshape(FCH, 128).T).astype(np.float32)
    w2_ext = bf16c(W2p.T)
    b2_ext = np.ascontiguousarray(b2adj.reshape(O, 1)).astype(np.float32)

    times = np.arange(T, dtype=np.float32) / np.float32(100.0)

    in_maps = []
    for k in range(NCORES):
        sl = slice(k * BL, (k + 1) * BL)
        a_loc = a[:, sl, :]
        x = np.empty((KX, T * BL), np.float32)
        x[:A] = a_loc.transpose(2, 0, 1).reshape(A, T * BL)
        x[A] = 1.0
        x[A + 1] = np.repeat(times, BL)
        h0 = init[sl].T
        h0r = np.ascontiguousarray(
            h0.reshape(LCH, 128, BL).transpose(1, 0, 2).reshape(128, LCH * BL)
        )
        in_maps.append(
            {
                "x_ext": bf16c(x),
                "wx_ext": wx_ext,
                "wh_ext": wh_ext,
                "h0_ext": bf16c(2.0 * h0r),                 # h2 = 2h
                "c0_ext": (2.0 * h0r).astype(np.float32),   # Q = 2c
                "w1_ext": w1_ext,
                "b1_ext": b1_ext,
                "w2_ext": w2_ext,
                "b2_ext": b2_ext,
            }
        )
    return in_maps


def kernel(init, a, s, W_ih, W_hh, b_ih, b_hh, W1, b1, W2, b2, **_):
    init = np.asarray(init, np.float32)
    a = np.asarray(a, np.float32)
    args = [np.asarray(x, np.float32) for x in (W_ih, W_hh, b_ih, b_hh, W1, b1, W2, b2)]
    in_maps = prep_inputs(init, a, *args)

    if "nc" not in _CACHE:
        _CACHE["nc"] = build_bass()
    nc = _CACHE["nc"]

    out = run_bass_kernel_spmd(nc, in_maps, list(range(NCORES)))
    _CACHE["last_result"] = out
    res = out.results

    x_hat = np.empty((T, B, O), np.float32)
    hs = np.empty((T, B, L), np.float32)
    for k in range(NCORES):
        sl = slice(k * BL, (k + 1) * BL)
        hs[:, sl, :] = res[k]["hs_out"].transpose(0, 2, 1)
        x_hat[:, sl, :] = res[k]["xh_out"].transpose(0, 2, 1)
    return x_hat, hs
